# revision 31
# baseline (speedup 1.0000x reference)
"""KMeans vq_codebook step on 8 NeuronCores (Trainium2, Bass/Tile).

Data-parallel over N: each core gets x/y shard [8192, 512]/[8192], centers
replicated. s' = x@c.T - c2/2 per 128-point tile, fp8e4m3 DoubleRow matmuls
(2x contraction per pass); -c2/2 seeded into PSUM by an f32r identity matmul
(exact); row max via DVE max8 from PSUM; argmax one-hot mask = Exp(SC*(s'-m))
on ACT (winner == 1.0 exactly); counts histogram = onehot(y).T @ mask in PSUM
via fp8 DoubleRow matmuls over tile PAIRS. The x transposes stage INSIDE the
ps tile's first bank (cast to fp8 before the seed overwrites it), freeing
enough PSUM for a 3-deep ps ring so the max8+exp drain of tile t-1 never
blocks the matmuls of tile t+1. Host: loss = sum(x2) - 2*sum(m), counts
all-reduce + per-cluster label argmax.
"""
import sys

sys.path.insert(0, "/opt/trn_rl_repo")

import numpy as np

import concourse.bass as bass
import concourse.mybir as mybir
from concourse import bacc
from concourse.bass import ds, ts
from concourse.bass_utils import run_bass_kernel_spmd
from concourse.masks import make_identity
from concourse.tile import TileContext

dt = mybir.dt
F32 = dt.float32
F32R = dt.float32r
BF16 = dt.bfloat16
FP8 = dt.float8e4
I32 = dt.int32
AF = mybir.ActivationFunctionType
ALU = mybir.AluOpType
DR = mybir.MatmulPerfMode.DoubleRow

N, D, K, NCLS, NCORES = 65536, 512, 1024, 10, 8
NSH = N // NCORES          # 8192 points per core
PT = NSH // 128            # 64 point-tiles per core
NPAIR = PT // 2            # 32 tile pairs
DC = D // 128              # 4 contraction chunks
SC = 1024.0                # exp sharpness for the argmax mask


def _build():
    nc = bacc.Bacc(None, target_bir_lowering=False, debug=False)
    x_in = nc.dram_tensor("x", [NSH, D], F32R, kind="ExternalInput")
    c_in = nc.dram_tensor("centers", [K, D], F32R, kind="ExternalInput")
    y_in = nc.dram_tensor("y", [NSH], I32, kind="ExternalInput")
    counts_out = nc.dram_tensor("counts", [16, K], F32, kind="ExternalOutput")
    loss_out = nc.dram_tensor("loss", [128, 2], F32, kind="ExternalOutput")
    scr = nc.dram_tensor("scr", [K // 128, 128], F32)  # c2 col->row bounce

    with TileContext(nc) as tc:
        with (
            tc.tile_pool(name="persist", bufs=1) as pp,
            tc.tile_pool(name="work", bufs=3) as wp,
            tc.tile_pool(name="prep", bufs=8) as prp,
            tc.tile_pool(name="psA", bufs=2, space="PSUM") as psA,   # s tiles
            tc.tile_pool(name="psB", bufs=2, space="PSUM") as psB,   # transpose staging
            tc.tile_pool(name="psH", bufs=1, space="PSUM") as psH,   # histogram
        ):
            ident = pp.tile([128, 128], F32)
            make_identity(nc, ident[:])
            identr = pp.tile([128, 128], F32R)
            nc.vector.tensor_copy(identr[:], ident[:])

            def xfetch(pr):
                t_ = wp.tile([128, 2, D], F32R, tag="xp")
                nc.sync.dma_start(
                    out=t_[:],
                    in_=x_in[ds(pr * 256, 256), :].rearrange(
                        "(two p) d -> p two d", two=2))
                return t_
            xtiles = {}

            # ---- prep: centers -> cT2 fp8 [d,dc,k]; c2 -> -c2/2 replicated
            cT2 = pp.tile([128, DC, K], FP8)
            c2cols = pp.tile([128, K // 128], F32)
            sqc = pp.tile([128, D], F32)
            stgs = {}
            for kc in range(K // 128):
                ct = prp.tile([128, D], F32R, tag="ct")
                nc.sync.dma_start(out=ct[:], in_=c_in[ts(kc, 128), :])
                nc.scalar.activation(sqc[:], ct[:], AF.Square,
                                     accum_out=c2cols[:, kc:kc + 1])
                stg = psB.tile([128, D], F32R, tag="tp")
                stgs[kc] = stg
                for dc in range(DC):
                    nc.tensor.matmul(stg[:, ts(dc, 128)],
                                     ct[:, ts(dc, 128)],
                                     identr[:], is_transpose=True,
                                     start=(dc == 0), stop=(dc == DC - 1),
                                     skip_group_check=True)
                nc.vector.tensor_copy(
                    cT2[:, :, ts(kc, 128)],
                    stg[:].rearrange("p (a b) -> p a b", a=DC))
            for _pr in range(2):
                xtiles[_pr] = xfetch(_pr)
            # c2 columns -> one 1024-wide row (via DRAM bounce), then -c2/2
            # replicated across partitions for the identity seed matmul.
            c2tp = psA.tile([128, K], F32, tag="ps")
            nc.tensor.matmul(c2tp[0:K // 128, 0:128], c2cols[:],
                             ident[:], is_transpose=True,
                             skip_group_check=True)
            c2row8 = pp.tile([K // 128, 128], F32)
            nc.vector.tensor_copy(c2row8[:], c2tp[0:K // 128, 0:128])
            nc.gpsimd.dma_start(out=scr[:, :], in_=c2row8[:])
            c2full = pp.tile([128, K], F32)
            nc.sync.dma_start(
                out=c2full[:],
                in_=scr[:, :].rearrange("k p -> () (k p)").to_broadcast((128, K)))
            negc2x = pp.tile([128, K], F32R)
            nc.vector.tensor_scalar_mul(negc2x[:], c2full[:], -0.5)

            # ---- y -> per-tile columns + one-hots, via fast contiguous DMA
            # and a PE transpose bounce (avoids an 8192x4B gather DMA).
            iota_i = pp.tile([128, 16], I32)
            nc.gpsimd.iota(iota_i[:], pattern=[[1, 16]], base=0, channel_multiplier=0)
            iota_f = pp.tile([128, 16], F32)
            nc.vector.tensor_copy(iota_f[:], iota_i[:])
            yrow_i = pp.tile([PT, 128], I32)
            nc.gpsimd.dma_start(out=yrow_i[:], in_=y_in[:].rearrange("(t p) -> t p", p=128))
            yrow = pp.tile([PT, 128], F32R)
            nc.vector.tensor_copy(yrow[:], yrow_i[:])
            ytp = psA.tile([128, K], F32, tag="ps")
            nc.tensor.matmul(ytp[:, 0:PT].bitcast(F32R),
                             yrow[:], identr[0:PT, 0:PT],
                             is_transpose=True, skip_group_check=True)
            ycol = pp.tile([128, PT], F32)
            nc.vector.tensor_copy(ycol[:], ytp[:, 0:PT])
            ohtfull = pp.tile([128, PT, 16], FP8)
            nc.vector.tensor_tensor(
                ohtfull[:],
                ycol[:, :, None].to_broadcast((128, PT, 16)),
                iota_f[:, None, :].to_broadcast((128, PT, 16)),
                ALU.is_equal)

            m8buf = pp.tile([128, PT * 16], F32)
            negm = pp.tile([128, PT], F32)
            hist = psH.tile([16, K], F32)

            # ---- PE warmup: ~4us of tiny matmuls right before the main GEMM
            # stream so the HAM clock-gate opens (cold K=4/8 halves PE clock).
            wt_f = pp.tile([128, 128], F32)
            nc.vector.memset(wt_f[:], 0.0)
            wt = wt_f[:].bitcast(BF16)[:, 0:128]
            wps = psA.tile([128, K], F32, tag="ps")
            for _ in range(40):
                nc.tensor.matmul(wps[:, 0:128], wt, wt,
                                 start=True, stop=True, skip_group_check=True)

            # ---- main loop; transposes staged one tile ahead INSIDE ps bank0
            pstiles = {}
            xTtiles = {}
            mptiles = {}

            def pair_setup(pr):
                if pr + 2 < NPAIR:
                    xtiles[pr + 2] = xfetch(pr + 2)
                mp = wp.tile([128, 2, K], FP8, tag="mp")
                mptiles[pr] = mp

            def transpose_cast(t):
                pr, i = divmod(t, 2)
                if i == 0:
                    pair_setup(pr)
                xpair = xtiles[pr]
                stg = psB.tile([128, D], F32R, tag="tp")
                for dc in range(DC):
                    nc.tensor.matmul(stg[:, ts(dc, 128)],
                                     xpair[:, i, ts(dc, 128)],
                                     identr[:], is_transpose=True,
                                     start=(dc == 0), stop=(dc == DC - 1),
                                     skip_group_check=True)
                xT = wp.tile([128, DC, 128], FP8, tag="xT")
                xTtiles[t] = xT
                nc.scalar.copy(xT[:].rearrange("p a b -> p (a b)"), stg[:])

            def gemm(t):
                ps = psA.tile([128, K], F32, tag="ps")
                pstiles[t] = ps
                xT = xTtiles.pop(t)
                # DR matmuls first (start), the -c2/2 seed accumulates LAST so
                # the first tiles don't stall on the prep-phase c2 chain.
                for g in range(2):
                    for kh in range(2):
                        nc.tensor.matmul(
                            ps[:, ds(kh * 512, 512)],
                            xT[:, ds(2 * g, 2), :],
                            cT2[:, ds(2 * g, 2), ds(kh * 512, 512)],
                            start=(g == 0), stop=False,
                            perf_mode=DR, skip_group_check=True)
                for kh in range(2):
                    nc.tensor.matmul(ps[:, ds(kh * 512, 512)], identr[:],
                                     negc2x[:, ds(kh * 512, 512)],
                                     start=False, stop=True,
                                     skip_group_check=True)

            def epilogue(t):
                ps = pstiles.pop(t)
                pr, i = divmod(t, 2)
                nc.scalar.activation(mptiles[pr][:, i, :], ps[:], AF.Exp,
                                     bias=negm[:, t:t + 1], scale=SC)

            def hist_mm(pr, start, stop):
                mp = mptiles.pop(pr)
                for kh in range(2):
                    nc.tensor.matmul(hist[:, ds(kh * 512, 512)],
                                     ohtfull[:, ds(2 * pr, 2), :],
                                     mp[:, :, ds(kh * 512, 512)],
                                     start=start, stop=stop,
                                     perf_mode=DR, skip_group_check=True)

            transpose_cast(0)
            for t in range(PT):
                pr, i = divmod(t, 2)
                if i == 1 and pr >= 2:
                    hist_mm(pr - 2, start=(pr == 2), stop=False)
                    if pr == NPAIR - 1:
                        hist_mm(pr - 1, start=False, stop=False)
                if t + 1 < PT:
                    transpose_cast(t + 1)
                if t >= 1:
                    epilogue(t - 1)
                gemm(t)
                for kh in range(2):
                    nc.vector.max(m8buf[:, ds(t * 16 + kh * 8, 8)],
                                  pstiles[t][:, ds(kh * 512, 512)])
                nc.vector.tensor_scalar(
                    out=negm[:, t:t + 1], in0=m8buf[:, t * 16:t * 16 + 1],
                    scalar1=m8buf[:, t * 16 + 8:t * 16 + 9], scalar2=-SC,
                    op0=ALU.max, op1=ALU.mult)
            epilogue(PT - 1)
            hist_mm(NPAIR - 1, start=False, stop=True)

            # ---- tail: loss partials + counts to DRAM
            lossb = pp.tile([128, 2], F32)
            nc.vector.memset(lossb[:], 0.0)
            nc.vector.tensor_reduce(lossb[:, 1:2], negm[:], axis=mybir.AxisListType.X,
                                    op=ALU.add)
            nc.sync.dma_start(out=loss_out[:], in_=lossb[:])
            csb = pp.tile([16, K], F32)
            for kh in range(2):
                nc.scalar.copy(csb[:, ds(kh * 512, 512)],
                               hist[:, ds(kh * 512, 512)])
                nc.sync.dma_start(out=counts_out[:, ds(kh * 512, 512)],
                                  in_=csb[:, ds(kh * 512, 512)])

    nc.finalize()
    return nc


_NC_CACHE: dict = {}


def _get_nc():
    if "nc" not in _NC_CACHE:
        _NC_CACHE["nc"] = _build()
    return _NC_CACHE["nc"]


def kernel(x, centers, y, _trace=False):
    x = np.ascontiguousarray(np.asarray(x, dtype=np.float32))
    centers = np.ascontiguousarray(np.asarray(centers, dtype=np.float32))
    y = np.ascontiguousarray(np.asarray(y, dtype=np.int32))
    nc = _get_nc()
    in_maps = [
        {"x": x[c * NSH:(c + 1) * NSH], "centers": centers,
         "y": y[c * NSH:(c + 1) * NSH]}
        for c in range(NCORES)
    ]
    res = run_bass_kernel_spmd(nc, in_maps, core_ids=list(range(NCORES)),
                               trace=_trace)
    counts = np.zeros((16, K), np.float64)
    loss = float((x.astype(np.float64) ** 2).sum())
    for r in res.results:
        counts += r["counts"].astype(np.float64)
        loss += (2.0 / SC) * r["loss"][:, 1].astype(np.float64).sum()
    correct = counts[:NCLS].max(axis=0).sum()
    acc = np.float32(correct / N)
    out = (np.float32(loss), acc)
    if _trace:
        return out, res
    return out


# revision 32
# speedup vs baseline: 1.1212x; 1.1212x over previous
"""KMeans vq_codebook step on 8 NeuronCores (Trainium2, Bass/Tile).

Data-parallel over N: each core gets x/y shard [8192, 512]/[8192], centers
replicated. s' = x@c.T - c2/2 per 128-point tile, fp8e4m3 DoubleRow matmuls
(2x contraction per pass); -c2/2 seeded into PSUM by an f32r identity matmul
(exact); row max via DVE max8 from PSUM; argmax one-hot mask = Exp(SC*(s'-m))
on ACT (winner == 1.0 exactly); counts histogram = onehot(y).T @ mask in PSUM
via fp8 DoubleRow matmuls over tile PAIRS. The x transposes stage INSIDE the
ps tile's first bank (cast to fp8 before the seed overwrites it), freeing
enough PSUM for a 3-deep ps ring so the max8+exp drain of tile t-1 never
blocks the matmuls of tile t+1. Host: loss = sum(x2) - 2*sum(m), counts
all-reduce + per-cluster label argmax.
"""
import sys

sys.path.insert(0, "/opt/trn_rl_repo")

import numpy as np

import concourse.bass as bass
import concourse.mybir as mybir
from concourse import bacc
from concourse.bass import ds, ts
from concourse.bass_utils import run_bass_kernel_spmd
from concourse.masks import make_identity
from concourse.tile import TileContext

dt = mybir.dt
F32 = dt.float32
F32R = dt.float32r
BF16 = dt.bfloat16
FP8 = dt.float8e4
I32 = dt.int32
AF = mybir.ActivationFunctionType
ALU = mybir.AluOpType
DR = mybir.MatmulPerfMode.DoubleRow

N, D, K, NCLS, NCORES = 65536, 512, 1024, 10, 8
NSH = N // NCORES          # 8192 points per core
PT = NSH // 128            # 64 point-tiles per core
NPAIR = PT // 2            # 32 tile pairs
DC = D // 128              # 4 contraction chunks
SC = 1024.0                # exp sharpness for the argmax mask


def _build():
    nc = bacc.Bacc(None, target_bir_lowering=False, debug=False)
    x_in = nc.dram_tensor("x", [NSH, D], F32R, kind="ExternalInput")
    c_in = nc.dram_tensor("centers", [K, D], F32R, kind="ExternalInput")
    y_in = nc.dram_tensor("y", [NSH], I32, kind="ExternalInput")
    counts_out = nc.dram_tensor("counts", [16, K], F32, kind="ExternalOutput")
    loss_out = nc.dram_tensor("loss", [128, 2], F32, kind="ExternalOutput")
    scr = nc.dram_tensor("scr", [K // 128, 128], F32)  # c2 col->row bounce

    with TileContext(nc) as tc:
        with (
            tc.tile_pool(name="persist", bufs=1) as pp,
            tc.tile_pool(name="work", bufs=3) as wp,
            tc.tile_pool(name="prep", bufs=8) as prp,
            tc.tile_pool(name="psA", bufs=2, space="PSUM") as psA,   # s tiles
            tc.tile_pool(name="psB", bufs=2, space="PSUM") as psB,   # transpose staging
            tc.tile_pool(name="psH", bufs=1, space="PSUM") as psH,   # histogram
        ):
            ident = pp.tile([128, 128], F32)
            make_identity(nc, ident[:])
            identr = pp.tile([128, 128], F32R)
            nc.vector.tensor_copy(identr[:], ident[:])

            def xfetch(pr):
                t_ = wp.tile([128, 2, D], F32R, tag="xp")
                nc.sync.dma_start(
                    out=t_[:],
                    in_=x_in[ds(pr * 256, 256), :].rearrange(
                        "(two p) d -> p two d", two=2))
                return t_
            xtiles = {}

            # ---- prep: centers -> cT2 fp8 [d,dc,k]; c2 -> -c2/2 replicated
            cT2 = pp.tile([128, DC, K], FP8)
            c2cols = pp.tile([128, K // 128], F32)
            sqc = pp.tile([128, D], F32)
            stgs = {}
            for kc in range(K // 128):
                ct = prp.tile([128, D], F32R, tag="ct")
                nc.sync.dma_start(out=ct[:], in_=c_in[ts(kc, 128), :])
                nc.scalar.activation(sqc[:], ct[:], AF.Square,
                                     accum_out=c2cols[:, kc:kc + 1])
                stg = psB.tile([128, D], F32R, tag="tp")
                stgs[kc] = stg
                for dc in range(DC):
                    nc.tensor.matmul(stg[:, ts(dc, 128)],
                                     ct[:, ts(dc, 128)],
                                     identr[:], is_transpose=True,
                                     start=(dc == 0), stop=(dc == DC - 1),
                                     skip_group_check=True)
                nc.vector.tensor_copy(
                    cT2[:, :, ts(kc, 128)],
                    stg[:].rearrange("p (a b) -> p a b", a=DC))
            for _pr in range(2):
                xtiles[_pr] = xfetch(_pr)
            # c2 columns -> one 1024-wide row (via DRAM bounce), then -c2/2
            # replicated across partitions for the identity seed matmul.
            c2tp = psA.tile([128, K], F32, tag="ps")
            nc.tensor.matmul(c2tp[0:K // 128, 0:128], c2cols[:],
                             ident[:], is_transpose=True,
                             skip_group_check=True)
            c2row8 = pp.tile([K // 128, 128], F32)
            nc.vector.tensor_copy(c2row8[:], c2tp[0:K // 128, 0:128])
            nc.gpsimd.dma_start(out=scr[:, :], in_=c2row8[:])
            c2full = pp.tile([128, K], F32)
            nc.sync.dma_start(
                out=c2full[:],
                in_=scr[:, :].rearrange("k p -> () (k p)").to_broadcast((128, K)))
            negc2x = pp.tile([128, K], F32R)
            nc.vector.tensor_scalar_mul(negc2x[:], c2full[:], -0.5)

            # ---- y -> per-tile columns + one-hots, via fast contiguous DMA
            # and a PE transpose bounce (avoids an 8192x4B gather DMA).
            iota_i = pp.tile([128, 16], I32)
            nc.gpsimd.iota(iota_i[:], pattern=[[1, 16]], base=0, channel_multiplier=0)
            iota_f = pp.tile([128, 16], F32)
            nc.vector.tensor_copy(iota_f[:], iota_i[:])
            yrow_i = pp.tile([PT, 128], I32)
            nc.gpsimd.dma_start(out=yrow_i[:], in_=y_in[:].rearrange("(t p) -> t p", p=128))
            yrow = pp.tile([PT, 128], F32R)
            nc.vector.tensor_copy(yrow[:], yrow_i[:])
            ytp = psA.tile([128, K], F32, tag="ps")
            nc.tensor.matmul(ytp[:, 0:PT].bitcast(F32R),
                             yrow[:], identr[0:PT, 0:PT],
                             is_transpose=True, skip_group_check=True)
            ycol = pp.tile([128, PT], F32)
            nc.vector.tensor_copy(ycol[:], ytp[:, 0:PT])
            ohtfull = pp.tile([128, PT, 16], FP8)
            nc.vector.tensor_tensor(
                ohtfull[:],
                ycol[:, :, None].to_broadcast((128, PT, 16)),
                iota_f[:, None, :].to_broadcast((128, PT, 16)),
                ALU.is_equal)

            m8buf = pp.tile([128, PT * 8], F32)
            negm = pp.tile([128, PT], F32)
            hist = psH.tile([16, K], F32)

            # ---- PE warmup: ~4us of tiny matmuls right before the main GEMM
            # stream so the HAM clock-gate opens (cold K=4/8 halves PE clock).
            wt_f = pp.tile([128, 128], F32)
            nc.vector.memset(wt_f[:], 0.0)
            wt = wt_f[:].bitcast(BF16)[:, 0:128]
            wps = psA.tile([128, K], F32, tag="ps")
            for _ in range(40):
                nc.tensor.matmul(wps[:, 0:128], wt, wt,
                                 start=True, stop=True, skip_group_check=True)

            # ---- main loop; transposes staged one tile ahead INSIDE ps bank0
            pstiles = {}
            xTtiles = {}
            mptiles = {}

            def pair_setup(pr):
                if pr + 2 < NPAIR:
                    xtiles[pr + 2] = xfetch(pr + 2)
                mp = wp.tile([128, 2, K], FP8, tag="mp")
                mptiles[pr] = mp

            def transpose_cast(t):
                pr, i = divmod(t, 2)
                if i == 0:
                    pair_setup(pr)
                xpair = xtiles[pr]
                stg = psB.tile([128, D], F32R, tag="tp")
                for dc in range(DC):
                    nc.tensor.matmul(stg[:, ts(dc, 128)],
                                     xpair[:, i, ts(dc, 128)],
                                     identr[:], is_transpose=True,
                                     start=(dc == 0), stop=(dc == DC - 1),
                                     skip_group_check=True)
                xT = wp.tile([128, DC, 128], FP8, tag="xT")
                xTtiles[t] = xT
                nc.scalar.copy(xT[:].rearrange("p a b -> p (a b)"), stg[:])

            def gemm(t):
                ps = psA.tile([128, K], F32, tag="ps")
                pstiles[t] = ps
                xT = xTtiles.pop(t)
                # DR matmuls first (start), the -c2/2 seed accumulates LAST so
                # the first tiles don't stall on the prep-phase c2 chain.
                for g in range(2):
                    for kh in range(2):
                        nc.tensor.matmul(
                            ps[:, ds(kh * 512, 512)],
                            xT[:, ds(2 * g, 2), :],
                            cT2[:, ds(2 * g, 2), ds(kh * 512, 512)],
                            start=(g == 0), stop=False,
                            perf_mode=DR, skip_group_check=True)
                for kh in range(2):
                    nc.tensor.matmul(ps[:, ds(kh * 512, 512)], identr[:],
                                     negc2x[:, ds(kh * 512, 512)],
                                     start=False, stop=True,
                                     skip_group_check=True)

            def epilogue(t):
                ps = pstiles.pop(t)
                pr, i = divmod(t, 2)
                nc.gpsimd.tensor_scalar_mul(negm[:, t:t + 1],
                                            m8buf[:, t * 8:t * 8 + 1], -SC)
                nc.scalar.activation(mptiles[pr][:, i, :], ps[:], AF.Exp,
                                     bias=negm[:, t:t + 1], scale=SC)

            def hist_mm(pr, start, stop):
                mp = mptiles.pop(pr)
                for kh in range(2):
                    nc.tensor.matmul(hist[:, ds(kh * 512, 512)],
                                     ohtfull[:, ds(2 * pr, 2), :],
                                     mp[:, :, ds(kh * 512, 512)],
                                     start=start, stop=stop,
                                     perf_mode=DR, skip_group_check=True)

            transpose_cast(0)
            for t in range(PT):
                pr, i = divmod(t, 2)
                if i == 1 and pr >= 2:
                    hist_mm(pr - 2, start=(pr == 2), stop=False)
                if t + 1 < PT:
                    transpose_cast(t + 1)
                if t >= 1:
                    epilogue(t - 1)
                gemm(t)
                nc.vector.max(m8buf[:, ts(t, 8)], pstiles[t][:])
            epilogue(PT - 1)
            hist_mm(NPAIR - 2, start=False, stop=False)
            hist_mm(NPAIR - 1, start=False, stop=True)

            # ---- tail: loss partials + counts to DRAM
            lossb = pp.tile([128, 2], F32)
            nc.vector.memset(lossb[:], 0.0)
            nc.vector.tensor_reduce(lossb[:, 1:2], negm[:], axis=mybir.AxisListType.X,
                                    op=ALU.add)
            nc.sync.dma_start(out=loss_out[:], in_=lossb[:])
            csb = pp.tile([16, K], F32)
            for kh in range(2):
                nc.scalar.copy(csb[:, ds(kh * 512, 512)],
                               hist[:, ds(kh * 512, 512)])
                nc.sync.dma_start(out=counts_out[:, ds(kh * 512, 512)],
                                  in_=csb[:, ds(kh * 512, 512)])

    nc.finalize()
    return nc


_NC_CACHE: dict = {}


def _get_nc():
    if "nc" not in _NC_CACHE:
        _NC_CACHE["nc"] = _build()
    return _NC_CACHE["nc"]


def kernel(x, centers, y, _trace=False):
    x = np.ascontiguousarray(np.asarray(x, dtype=np.float32))
    centers = np.ascontiguousarray(np.asarray(centers, dtype=np.float32))
    y = np.ascontiguousarray(np.asarray(y, dtype=np.int32))
    nc = _get_nc()
    in_maps = [
        {"x": x[c * NSH:(c + 1) * NSH], "centers": centers,
         "y": y[c * NSH:(c + 1) * NSH]}
        for c in range(NCORES)
    ]
    res = run_bass_kernel_spmd(nc, in_maps, core_ids=list(range(NCORES)),
                               trace=_trace)
    counts = np.zeros((16, K), np.float64)
    loss = float((x.astype(np.float64) ** 2).sum())
    for r in res.results:
        counts += r["counts"].astype(np.float64)
        loss += (2.0 / SC) * r["loss"][:, 1].astype(np.float64).sum()
    correct = counts[:NCLS].max(axis=0).sum()
    acc = np.float32(correct / N)
    out = (np.float32(loss), acc)
    if _trace:
        return out, res
    return out
